# revision 14
# baseline (speedup 1.0000x reference)
"""Dense GAT layer (nn_DenseGATLayer) Trainium2 Bass kernel, v3.

Problem (per batch b of B=8):
    Wh   = X[b] @ W                                   [N=1024, H*F=256]
    s[n,h] = <Wh[n,h,:], a_src[h]>,  d[n,h] = <Wh[n,h,:], a_dst[h]>
    e[i,j,h] = lrelu(s[i,h] + d[j,h], 0.2);  masked by A[b,i,j]
    alpha = softmax_j(e);  out[i,h,:] = elu(sum_j alpha[i,j,h] Wh[j,h,:])

Sharding: data-parallel, one batch per NeuronCore (B=8 == n_cores=8).

The N^2*H masked-softmax-numerator work (32 tiles of [128j, 1024i]) is
the whole game; custom DVE ops run at 1 row/cycle regardless of dtype,
so one engine cannot carry it. v3 splits it across ALL engines using
two formulations:

  separable:  exp(lrelu(s_i+d_j)) = max(e^s e^d, e^{0.2s} e^{0.2d})
  log-space:  P = exp(max(t, 0.2t)/S), t = S(s_i+d_j) + m   (v1 path)

with an ADDITIVE mask m = ATm[j,i] = (A^T-1)*1e6 in {0, -1e6} (bf16,
host-prepped). Per-head modes:

  h0 T5:    PE builds w' = v (x) u + ATm and z' = q (x) p + ATm in PSUM
            (K=1 outer products + identity-stationary mask replicate);
            GpSimd scalar_tensor_tensor emits P = max(max(w',0), z').
            Zero DVE, zero Scalar.
  h1,h2 M_EXP: DVE score op t = ATm + (s16 + d16); e = max(t, .2t)
            (int16 fixed-point, scale 2048); Scalar exp -> bf16 P.
  h3 M_GP:  GpSimd stt G = p_rep*q_j + ATm; DVE select
            P = G>0 ? max(u_rep*v_j, G) : 0.

Host prep: X^T, ATm, and Wa = W @ blockdiag(a) are precomputed; fp32
matmuls use fp32r moving operands (1 cycle/row). Row vectors (u,p,v,q)
ride a DRAM round trip: stride-0 broadcast DMAs replicate them across
partitions where needed.
"""

import sys

if "/opt/trn_rl_repo" not in sys.path:
    sys.path.insert(0, "/opt/trn_rl_repo")

from contextlib import ExitStack

import numpy as np

import concourse.bass as bass
import concourse.tile as tile
from concourse import bacc, mybir
from concourse import bass_utils
from concourse._compat import with_exitstack

# ------------------------------------------------------------------ params
B, N, DIN, H, F = 8, 1024, 256, 4, 64
HF = H * F
NT = N // 128            # 8 node tiles
KT = DIN // 128          # 2 contraction tiles
LRELU_ALPHA = 0.2
SCALE = 2048.0           # fixed-point scale for the log-space path
MBIG = 1.0e6             # additive mask magnitude (host bakes -MBIG into ATm)

dt = mybir.dt
AF = mybir.ActivationFunctionType
f32r = dt.float32r
ALU = mybir.AluOpType

# ------------------------------------------------------------- custom DVE ops
from concourse.dve_ops import (
    DveOp,
    OPS,
    _SUB_OPCODE_FOR_NAME,
    CUSTOM_DVE_SPECS,
    _CUSTOM_DVE_ROW_BASE,
)
from concourse.dve_spec import (
    Spec,
    Src0,
    Src1,
    C0,
    C1,
    C2,
    Zero,
    One,
    lower,
    maxx,
    select,
    _has_src1,
)
from concourse.dve_uop import DveOpSpec


def _register_op(name, spec):
    for o in OPS:
        if o.name == name:
            return o
    opcode = _CUSTOM_DVE_ROW_BASE + len(OPS)
    shas = {}
    for ver in ("v3", "v4"):
        s = DveOpSpec(
            name=name, opcode=opcode, uops=lower(spec, ver=ver), rd1_en=_has_src1(spec)
        )
        shas[ver] = s.sha(ver)
    op = DveOp(name, spec, subdim=False, uops_sha=shas)
    OPS.append(op)
    _SUB_OPCODE_FOR_NAME[name] = opcode
    CUSTOM_DVE_SPECS[name] = spec
    return op


def _score3_ref(in0, in1, s0, s1, imm2):
    t = np.asarray(in0, np.float32) * imm2 + np.asarray(in1, np.float32) + s0
    return np.maximum(t, t * s1)


# log-space masked leaky-relu score with int8 mask (in0: 1 = masked-out):
# t = in0*imm2 + in1 + s0; out = max(t, t*s1)
_t = Src0 * C2 + (Src1 + C0)
GAT_SCORE3 = _register_op(
    "GAT_SCORE3_ANT", Spec(body=maxx(_t, _t * C1), reference=_score3_ref)
)

# separable passB: P = in1 > 0 ? max(in0 * s0, in1) : 0
GAT_WMAX = _register_op(
    "GAT_WMAX_ANT",
    Spec(
        body=select(Src1 > Zero, maxx(Src0 * C0, Src1), Zero),
        reference=lambda in0, in1, s0, s1, imm2: np.where(
            np.asarray(in1, np.float32) > 0,
            np.maximum(np.asarray(in0, np.float32) * s0, np.asarray(in1, np.float32)),
            0.0,
        ),
    ),
)

# elu select: out = in0 >= 0 ? in0 : in1 - 1
GAT_SEL2 = _register_op(
    "GAT_SEL2_ANT",
    Spec(
        body=select(Src0 >= Zero, Src0, Src1 - One),
        reference=lambda in0, in1, s0, s1, imm2: np.where(in0 >= 0, in0, in1 - 1),
    ),
)


def _bcast_last(ap, n):
    """Append a step-0 free dim of size n to an AP (broadcast along it)."""
    return bass.AP(ap.tensor, ap.offset, [list(d) for d in ap.ap] + [[0, n]])


def _bcast_part(ap, n):
    """Prepend a step-0 partition dim of size n (broadcast; DMA use only)."""
    return bass.AP(ap.tensor, ap.offset, [[0, n]] + [list(d) for d in ap.ap])


# per-(h, jt) pipeline modes: h0 mixes pure-GpSimd (T6) and GpSimd+DVE
# (M_GP) tiles; h1-h3 run the log-space DVE+Scalar path (M_EXP).
MODE = {h: ["EXP"] * 8 for h in range(H)}
SREP = {0: 0, 1: 1, 2: 2, 3: 3}  # s16-replica slot per M_EXP head


# ------------------------------------------------------------------ kernel body
@with_exitstack
def _gat_body(ctx: ExitStack, tc: "tile.TileContext", XTd, ATd, Wd, Wad, ID16d, OUTd):
    nc = tc.nc
    f32, bf16, i16 = dt.float32, dt.bfloat16, dt.int16

    sb = ctx.enter_context(tc.tile_pool(name="sb", bufs=1))
    dram = ctx.enter_context(tc.tile_pool(name="dram", bufs=1, space="DRAM"))

    # ---------- input loads, earliest-needed first ----------------------------
    XTsb = sb.tile([128, KT * N], f32r)  # [p=din%128, kt, node]
    XTv = XTsb[:].rearrange("p (kt n) -> p kt n", kt=KT)
    XTdv = XTd[:].rearrange("(kt p) n -> p kt n", p=128)
    x_insts = []
    for nh in range(2):
        for kt in range(KT):
            eng = nc.sync if kt == 0 else nc.scalar
            xi = eng.dma_start(
                XTv[:, kt, nh * 512 : (nh + 1) * 512],
                XTdv[:, kt, nh * 512 : (nh + 1) * 512].bitcast(f32r),
            )
            x_insts.append(xi)

    ident = sb.tile([16, 16], f32)
    nc.sync.dma_start(ident[:], ID16d[:])
    Wsb = sb.tile([128, KT * HF], f32r)
    Wasb = sb.tile([128, KT * 2 * H], f32r)
    for kt in range(KT):
        nc.scalar.dma_start(
            Wsb[:, kt * HF : (kt + 1) * HF], Wd[kt * 128 : (kt + 1) * 128, :].bitcast(f32r)
        )
        nc.scalar.dma_start(
            Wasb[:, kt * 2 * H : (kt + 1) * 2 * H], Wad[kt * 128 : (kt + 1) * 128, :].bitcast(f32r)
        )

    ATsb = sb.tile([128, NT * N], dt.int8)  # A8 tile jt at cols [jt*N, (jt+1)*N)
    ATdv = ATd[:].rearrange("(jt p) n -> p jt n", p=128)
    for jt in range(NT):
        eng = nc.sync if jt % 2 == 0 else nc.scalar
        ai = eng.dma_start(ATsb[:, jt * N : (jt + 1) * N], ATdv[:, jt, :])
        for xi in x_insts:
            tile.add_dep_helper(ai.ins, xi.ins, reason="XT before AT")

    # warm the exp activation table off the critical path
    scrap = sb.tile([1, 1], f32)
    nc.gpsimd.memset(scrap[:], 0.0)
    nc.scalar.activation(scrap[:], scrap[:], AF.Exp)

    # ---------- score-vector prep ---------------------------------------------
    # eT rows (Wa col order): 2h = s_h, 2h+1 = d_h
    eT2048 = sb.tile([8, N], f32)    # 2048 * eT (for fixed-point d columns)
    s16d = sb.tile([8, N], i16)      # round(2048 * eT) int16 rows
    S16dr = dram.tile([8, N], i16)

    S16REP = sb.tile([128, H * N], i16)   # s16 replicated rows per head
    dcolsR = sb.tile([128, NT * 8], f32)   # raw 2048*eT cols (d16 at col 2h+1)
    Whb = sb.tile([128, NT * H * (F + 1)], bf16)  # [p=node, jt, h, f|1]
    w4 = Whb[:].rearrange("p (jt h f) -> p jt h f", jt=NT, h=H)

    with (
        tc.tile_pool(name="psE", bufs=1, space="PSUM") as psE,
        tc.tile_pool(name="psT", bufs=2, space="PSUM") as psT,
        tc.tile_pool(name="psW", bufs=2, space="PSUM") as psW,
    ):
        # eT = Wa^T @ X^T  ([8, N] fp32)
        pe = psE.tile([8, N], f32)
        eT_mms = []
        for nh in range(2):
            for kt in range(KT):
                mi = nc.tensor.matmul(
                    pe[:, nh * 512 : (nh + 1) * 512],
                    Wasb[:, kt * 2 * H : (kt + 1) * 2 * H],
                    XTsb[:, kt * N + nh * 512 : kt * N + (nh + 1) * 512],
                    start=(kt == 0),
                    stop=(kt == KT - 1),
                )
                eT_mms.append(mi)
        # fixed-point rows: s16 (int16) for broadcast; eT2048 for d columns.
        # Split by nh halves so the DRAM round trip starts as soon as the
        # first half of eT retires.
        s16_acts = []
        for nh in range(2):
            cs = slice(nh * 512, (nh + 1) * 512)
            ai = nc.scalar.activation(s16d[:, cs], pe[:, cs], AF.Copy, scale=SCALE)
            s16_acts.append(ai)
            nc.scalar.dma_start(S16dr[:, cs], s16d[:, cs])
        s16_acts.append(nc.scalar.activation(eT2048[:], pe[:], AF.Copy, scale=SCALE))

        # s16 rows replicated across partitions (stride-0 DRAM broadcast)
        for h, k in SREP.items():
            eng = nc.sync if k % 2 == 0 else nc.scalar
            eng.dma_start(
                S16REP[:, k * N : (k + 1) * N], _bcast_part(S16dr[2 * h, :], 128)
            )

        # d columns: PE transposes of 2048*eT 128-col slabs
        for jt in range(NT):
            pd = psT.tile([128, 8], f32, tag="pt")
            sl = slice(jt * 128, (jt + 1) * 128)
            nc.tensor.transpose(pd[:], eT2048[:, sl], ident[0:8, 0:8])
            nc.vector.tensor_copy(dcolsR[:, jt * 8 : (jt + 1) * 8], pd[:])

        # Wh (node-major bf16, strided per-head layout with ones column).
        # Explicit deps keep the critical eT -> s16 -> broadcast chain ahead
        # of Wh work in the in-order Tensor/Scalar queues.
        nc.gpsimd.memset(w4[:, :, :, F], 1.0)
        for it in range(NT):
            pw = psW.tile([128, HF], f32, tag="pw")
            for kt in range(KT):
                mi = nc.tensor.matmul(
                    pw[:],
                    XTsb[:, kt * N + it * 128 : kt * N + (it + 1) * 128],
                    Wsb[:, kt * HF : (kt + 1) * HF],
                    start=(kt == 0),
                    stop=(kt == KT - 1),
                )
                if it == 0:
                    for ei in eT_mms:
                        tile.add_dep_helper(mi.ins, ei.ins, reason="eT before Wh")
            ci = nc.scalar.copy(
                w4[:, it, :, 0:F], pw[:].rearrange("p (h f) -> p h f", h=H)
            )
            if it < 4:
                for si in s16_acts:
                    tile.add_dep_helper(ci.ins, si.ins, reason="s16 before w4 copies")

    # ---------- head loop ------------------------------------------------------
    OutSB = sb.tile([128, NT * HF], f32)
    Out4 = OutSB[:].rearrange("p (it h f) -> p it h f", it=NT, h=H)
    psA = ctx.enter_context(tc.tile_pool(name="psA", bufs=2, space="PSUM"))
    pts = ctx.enter_context(tc.tile_pool(name="pts", bufs=2))
    gps = ctx.enter_context(tc.tile_pool(name="gps", bufs=2))
    eps = ctx.enter_context(tc.tile_pool(name="eps", bufs=3))
    small = ctx.enter_context(tc.tile_pool(name="small", bufs=3))

    PThs = {}

    def emit_scores_M_EXP(h, PTh, srep_idx):
        expw = 1 if h == H - 1 else 2
        for jp in range(NT // expw):
            ep = eps.tile([128, expw * N], f32, tag="ep")
            for k in range(expw):
                jt = expw * jp + k
                nc.vector._custom_dve(
                    GAT_SCORE3,
                    out=ep[:, k * N : (k + 1) * N],
                    in0=ATsb[:, jt * N : (jt + 1) * N],
                    in1=S16REP[:, srep_idx * N : (srep_idx + 1) * N],
                    s0=dcolsR[:, jt * 8 + 2 * h + 1 : jt * 8 + 2 * h + 2],
                    s1=LRELU_ALPHA,
                    imm2=-MBIG,
                )
            nc.scalar.activation(
                PTh[:, jp * expw * N : (jp + 1) * expw * N],
                ep[:],
                AF.Exp,
                scale=1.0 / SCALE,
            )

    def emit_av_norm(h, PTh):
        acc8 = psA.tile([128, NT, 128], f32, tag="acc")
        for it in range(NT):
            for jt in range(NT):
                nc.tensor.matmul(
                    acc8[:, it, 0 : F + 1],
                    PTh[:, jt * N + it * 128 : jt * N + (it + 1) * 128],
                    w4[:, jt, h, :],
                    start=(jt == 0),
                    stop=(jt == NT - 1),
                )
        ngroups = 2
        gsz = NT // ngroups
        for g in range(ngroups):
            its = slice(g * gsz, (g + 1) * gsz)
            rz = small.tile([128, gsz], f32, tag="rz")
            nc.vector.reciprocal(rz[:], acc8[:, its, F])
            u8 = small.tile([128, gsz, F], f32, tag="u8")
            nc.vector.tensor_tensor(
                u8[:], acc8[:, its, 0:F], _bcast_last(rz[:], F), op=ALU.mult
            )
            ev8 = small.tile([128, gsz, F], f32, tag="ev8")
            nc.scalar.activation(ev8[:], u8[:], AF.Exp)
            nc.vector._custom_dve(GAT_SEL2, out=Out4[:, its, h, :], in0=u8[:], in1=ev8[:])

    OutV = OutSB[:].rearrange("p (it c) -> p it c", it=NT)
    OutDV = OUTd[:].rearrange("(it p) c -> p it c", p=128)
    for h in range(H):
        PTh = pts.tile([128, NT * N], bf16, tag="pt")
        PThs[h] = PTh
        emit_scores_M_EXP(h, PTh, SREP[h])
        emit_av_norm(h, PTh)
        if h == H - 1:
            # store each it-half as soon as the last head's norm group for
            # it completes (emit_av_norm emits g0 then g1)
            nc.sync.dma_start(OutDV[:, 0 : NT // 2, :], OutV[:, 0 : NT // 2, :])
            nc.scalar.dma_start(OutDV[:, NT // 2 : NT, :], OutV[:, NT // 2 : NT, :])


# ------------------------------------------------------------------ build/run
_NC_CACHE = {}


def _build_nc():
    if "nc" in _NC_CACHE:
        return _NC_CACHE["nc"]
    nc = bacc.Bacc(
        "TRN2",
        target_bir_lowering=False,
        debug=False,
        enable_asserts=False,
        num_devices=B,
    )
    XTd = nc.dram_tensor("XT", [DIN, N], dt.float32, kind="ExternalInput").ap()
    ATd = nc.dram_tensor("A8", [N, N], dt.int8, kind="ExternalInput").ap()
    Wd = nc.dram_tensor("W", [DIN, HF], dt.float32, kind="ExternalInput").ap()
    Wad = nc.dram_tensor("Wa", [DIN, 2 * H], dt.float32, kind="ExternalInput").ap()
    ID16d = nc.dram_tensor("ID16", [16, 16], dt.float32, kind="ExternalInput").ap()
    OUTd = nc.dram_tensor("OUT", [N, HF], dt.float32, kind="ExternalOutput").ap()
    with tile.TileContext(nc) as tc:
        _gat_body(tc, XTd, ATd, Wd, Wad, ID16d, OUTd)
    nc.compile()
    _NC_CACHE["nc"] = nc
    return nc


def _host_prep(W, a_src, a_dst):
    Wh_w = np.asarray(W, np.float32).reshape(DIN, H, F)
    Wa = np.empty((DIN, 2 * H), np.float32)
    Wa[:, 0::2] = np.einsum("khf,hf->kh", Wh_w, np.asarray(a_src, np.float32))
    Wa[:, 1::2] = np.einsum("khf,hf->kh", Wh_w, np.asarray(a_dst, np.float32))
    return Wa


def _run(X, A, W, a_src, a_dst, **spmd_kwargs):
    import ml_dtypes

    bf = ml_dtypes.bfloat16
    X = np.asarray(X, np.float32)
    XT = np.ascontiguousarray(X.transpose(0, 2, 1))                  # [B, DIN, N]
    A8 = np.ascontiguousarray(
        (1 - np.asarray(A, np.int32).transpose(0, 2, 1)).astype(np.int8)
    )                                                                 # [B, N(j), N(i)]
    W = np.ascontiguousarray(np.asarray(W, np.float32))
    Wa = _host_prep(W, a_src, a_dst)
    nc = _build_nc()
    id16 = np.eye(16, dtype=np.float32)
    in_maps = [
        {"XT": XT[b], "A8": A8[b], "W": W, "Wa": Wa, "ID16": id16}
        for b in range(B)
    ]
    res = bass_utils.run_bass_kernel_spmd(
        nc, in_maps, core_ids=list(range(B)), **spmd_kwargs
    )
    out = np.stack([np.asarray(res.results[b]["OUT"]) for b in range(B)])
    return out.astype(np.float32), res


def kernel(X, A, W, a_src, a_dst):
    out, _ = _run(X, A, W, a_src, a_dst)
    return out


if __name__ == "__main__":
    rng = np.random.default_rng(0)
    out = kernel(
        X=rng.standard_normal((B, N, DIN)).astype(np.float32),
        A=rng.integers(0, 2, size=(B, N, N)).astype(np.int32),
        W=(rng.standard_normal((DIN, HF)) * 0.06).astype(np.float32),
        a_src=(rng.standard_normal((H, F)) * 0.17).astype(np.float32),
        a_dst=(rng.standard_normal((H, F)) * 0.17).astype(np.float32),
    )
    print(out.shape, out.dtype)


# revision 18
# speedup vs baseline: 1.0274x; 1.0274x over previous
"""Dense GAT layer (nn_DenseGATLayer) Trainium2 Bass kernel, v3.

Problem (per batch b of B=8):
    Wh   = X[b] @ W                                   [N=1024, H*F=256]
    s[n,h] = <Wh[n,h,:], a_src[h]>,  d[n,h] = <Wh[n,h,:], a_dst[h]>
    e[i,j,h] = lrelu(s[i,h] + d[j,h], 0.2);  masked by A[b,i,j]
    alpha = softmax_j(e);  out[i,h,:] = elu(sum_j alpha[i,j,h] Wh[j,h,:])

Sharding: data-parallel, one batch per NeuronCore (B=8 == n_cores=8).

The N^2*H masked-softmax-numerator work (32 tiles of [128j, 1024i]) is
the whole game; custom DVE ops run at 1 row/cycle regardless of dtype,
so one engine cannot carry it. v3 splits it across ALL engines using
two formulations:

  separable:  exp(lrelu(s_i+d_j)) = max(e^s e^d, e^{0.2s} e^{0.2d})
  log-space:  P = exp(max(t, 0.2t)/S), t = S(s_i+d_j) + m   (v1 path)

with an ADDITIVE mask m = ATm[j,i] = (A^T-1)*1e6 in {0, -1e6} (bf16,
host-prepped). Per-head modes:

  h0 T5:    PE builds w' = v (x) u + ATm and z' = q (x) p + ATm in PSUM
            (K=1 outer products + identity-stationary mask replicate);
            GpSimd scalar_tensor_tensor emits P = max(max(w',0), z').
            Zero DVE, zero Scalar.
  h1,h2 M_EXP: DVE score op t = ATm + (s16 + d16); e = max(t, .2t)
            (int16 fixed-point, scale 2048); Scalar exp -> bf16 P.
  h3 M_GP:  GpSimd stt G = p_rep*q_j + ATm; DVE select
            P = G>0 ? max(u_rep*v_j, G) : 0.

Host prep: X^T, ATm, and Wa = W @ blockdiag(a) are precomputed; fp32
matmuls use fp32r moving operands (1 cycle/row). Row vectors (u,p,v,q)
ride a DRAM round trip: stride-0 broadcast DMAs replicate them across
partitions where needed.
"""

import sys

if "/opt/trn_rl_repo" not in sys.path:
    sys.path.insert(0, "/opt/trn_rl_repo")

from contextlib import ExitStack

import numpy as np

import concourse.bass as bass
import concourse.tile as tile
from concourse import bacc, mybir
from concourse import bass_utils
from concourse._compat import with_exitstack

# ------------------------------------------------------------------ params
B, N, DIN, H, F = 8, 1024, 256, 4, 64
HF = H * F
NT = N // 128            # 8 node tiles
KT = DIN // 128          # 2 contraction tiles
LRELU_ALPHA = 0.2
SCALE = 2048.0           # fixed-point scale for the log-space path
MBIG = 1.0e6             # additive mask magnitude (host bakes -MBIG into ATm)

dt = mybir.dt
AF = mybir.ActivationFunctionType
f32r = dt.float32r
ALU = mybir.AluOpType

# ------------------------------------------------------------- custom DVE ops
from concourse.dve_ops import (
    DveOp,
    OPS,
    _SUB_OPCODE_FOR_NAME,
    CUSTOM_DVE_SPECS,
    _CUSTOM_DVE_ROW_BASE,
)
from concourse.dve_spec import (
    Spec,
    Src0,
    Src1,
    C0,
    C1,
    C2,
    Zero,
    One,
    lower,
    maxx,
    select,
    _has_src1,
)
from concourse.dve_uop import DveOpSpec


def _register_op(name, spec):
    for o in OPS:
        if o.name == name:
            return o
    opcode = _CUSTOM_DVE_ROW_BASE + len(OPS)
    shas = {}
    for ver in ("v3", "v4"):
        s = DveOpSpec(
            name=name, opcode=opcode, uops=lower(spec, ver=ver), rd1_en=_has_src1(spec)
        )
        shas[ver] = s.sha(ver)
    op = DveOp(name, spec, subdim=False, uops_sha=shas)
    OPS.append(op)
    _SUB_OPCODE_FOR_NAME[name] = opcode
    CUSTOM_DVE_SPECS[name] = spec
    return op


def _score3_ref(in0, in1, s0, s1, imm2):
    t = np.asarray(in0, np.float32) * imm2 + np.asarray(in1, np.float32) + s0
    return np.maximum(t, t * s1)


# log-space masked leaky-relu score with int8 mask (in0: 1 = masked-out):
# t = in0*imm2 + in1 + s0; out = max(t, t*s1)
_t = Src0 * C2 + (Src1 + C0)
GAT_SCORE3 = _register_op(
    "GAT_SCORE3_ANT", Spec(body=maxx(_t, _t * C1), reference=_score3_ref)
)

# separable passB: P = in1 > 0 ? max(in0 * s0, in1) : 0
GAT_WMAX = _register_op(
    "GAT_WMAX_ANT",
    Spec(
        body=select(Src1 > Zero, maxx(Src0 * C0, Src1), Zero),
        reference=lambda in0, in1, s0, s1, imm2: np.where(
            np.asarray(in1, np.float32) > 0,
            np.maximum(np.asarray(in0, np.float32) * s0, np.asarray(in1, np.float32)),
            0.0,
        ),
    ),
)

# elu select: out = in0 >= 0 ? in0 : in1 - 1
GAT_SEL2 = _register_op(
    "GAT_SEL2_ANT",
    Spec(
        body=select(Src0 >= Zero, Src0, Src1 - One),
        reference=lambda in0, in1, s0, s1, imm2: np.where(in0 >= 0, in0, in1 - 1),
    ),
)


def _bcast_last(ap, n):
    """Append a step-0 free dim of size n to an AP (broadcast along it)."""
    return bass.AP(ap.tensor, ap.offset, [list(d) for d in ap.ap] + [[0, n]])


def _bcast_part(ap, n):
    """Prepend a step-0 partition dim of size n (broadcast; DMA use only)."""
    return bass.AP(ap.tensor, ap.offset, [[0, n]] + [list(d) for d in ap.ap])


# per-(h, jt) pipeline modes: h0 mixes pure-GpSimd (T6) and GpSimd+DVE
# (M_GP) tiles; h1-h3 run the log-space DVE+Scalar path (M_EXP).
MODE = {h: ["EXP"] * 8 for h in range(H)}
SREP = {0: 0, 1: 1, 2: 2, 3: 3}  # s16-replica slot per M_EXP head


# ------------------------------------------------------------------ kernel body
@with_exitstack
def _gat_body(ctx: ExitStack, tc: "tile.TileContext", XTd, ATd, Wd, Wad, ID16d, OUTd):
    nc = tc.nc
    f32, bf16, i16 = dt.float32, dt.bfloat16, dt.int16

    sb = ctx.enter_context(tc.tile_pool(name="sb", bufs=1))
    dram = ctx.enter_context(tc.tile_pool(name="dram", bufs=1, space="DRAM"))

    # ---------- input loads, earliest-needed first ----------------------------
    XTsb = sb.tile([128, KT * N], f32r)  # [p=din%128, kt, node]
    XTv = XTsb[:].rearrange("p (kt n) -> p kt n", kt=KT)
    XTdv = XTd[:].rearrange("(kt p) n -> p kt n", p=128)
    x_insts = [
        nc.sync.dma_start(XTv[:, 0, :], XTdv[:, 0, :].bitcast(f32r)),
        nc.scalar.dma_start(XTv[:, 1, :], XTdv[:, 1, :].bitcast(f32r)),
    ]

    ident = sb.tile([16, 16], f32)
    nc.sync.dma_start(ident[:], ID16d[:])
    Wsb = sb.tile([128, KT * HF], f32r)
    Wasb = sb.tile([128, KT * 2 * H], f32r)
    for kt in range(KT):
        nc.sync.dma_start(
            Wasb[:, kt * 2 * H : (kt + 1) * 2 * H], Wad[kt * 128 : (kt + 1) * 128, :].bitcast(f32r)
        )
        nc.sync.dma_start(
            Wsb[:, kt * HF : (kt + 1) * HF], Wd[kt * 128 : (kt + 1) * 128, :].bitcast(f32r)
        )

    ATsb = sb.tile([128, NT * N], dt.int8)  # A8 tile jt at cols [jt*N, (jt+1)*N)
    ATdv = ATd[:].rearrange("(jt p) n -> p jt n", p=128)

    # warm the exp activation table off the critical path
    scrap = sb.tile([1, 1], f32)
    nc.gpsimd.memset(scrap[:], 0.0)
    nc.scalar.activation(scrap[:], scrap[:], AF.Exp)

    # ---------- score-vector prep ---------------------------------------------
    # eT rows (Wa col order): 2h = s_h, 2h+1 = d_h
    eT2048 = sb.tile([8, N], f32)    # 2048 * eT (for fixed-point d columns)
    s16d = sb.tile([8, N], i16)      # round(2048 * eT) int16 rows
    S16dr = dram.tile([8, N], i16)

    S16REP = sb.tile([128, H * N], i16)   # s16 replicated rows per head
    dcolsR = sb.tile([128, NT * 8], f32)   # raw 2048*eT cols (d16 at col 2h+1)
    Whb = sb.tile([128, NT * H * (F + 1)], bf16)  # [p=node, jt, h, f|1]
    w4 = Whb[:].rearrange("p (jt h f) -> p jt h f", jt=NT, h=H)

    with (
        tc.tile_pool(name="psE", bufs=1, space="PSUM") as psE,
        tc.tile_pool(name="psT", bufs=2, space="PSUM") as psT,
        tc.tile_pool(name="psW", bufs=2, space="PSUM") as psW,
    ):
        # eT = Wa^T @ X^T  ([8, N] fp32)
        pe = psE.tile([8, N], f32)
        eT_mms = []
        for nh in range(2):
            for kt in range(KT):
                mi = nc.tensor.matmul(
                    pe[:, nh * 512 : (nh + 1) * 512],
                    Wasb[:, kt * 2 * H : (kt + 1) * 2 * H],
                    XTsb[:, kt * N + nh * 512 : kt * N + (nh + 1) * 512],
                    start=(kt == 0),
                    stop=(kt == KT - 1),
                )
                eT_mms.append(mi)
        # fixed-point rows: s16 (int16) for broadcast; eT2048 for d columns.
        # Split by nh halves so the DRAM round trip starts as soon as the
        # first half of eT retires.
        s16_acts = []
        for nh in range(2):
            cs = slice(nh * 512, (nh + 1) * 512)
            ai = nc.scalar.activation(s16d[:, cs], pe[:, cs], AF.Copy, scale=SCALE)
            s16_acts.append(ai)
            nc.scalar.dma_start(S16dr[:, cs], s16d[:, cs])
        s16_acts.append(nc.scalar.activation(eT2048[:], pe[:], AF.Copy, scale=SCALE))

        # s16 rows replicated across partitions (stride-0 DRAM broadcast)
        for h, k in SREP.items():
            eng = nc.sync if k % 2 == 0 else nc.scalar
            eng.dma_start(
                S16REP[:, k * N : (k + 1) * N], _bcast_part(S16dr[2 * h, :], 128)
            )

        # A8 tiles: emitted after the critical broadcast chain so their
        # HWDGE descriptor generation does not delay it; jt 0-3 on the
        # scalar queue land first for the first head's score stream.
        for jt in range(NT):
            eng = nc.scalar if jt < 4 else nc.sync
            eng.dma_start(ATsb[:, jt * N : (jt + 1) * N], ATdv[:, jt, :])

        # d columns: PE transposes of 2048*eT 128-col slabs
        for jt in range(NT):
            pd = psT.tile([128, 8], f32, tag="pt")
            sl = slice(jt * 128, (jt + 1) * 128)
            nc.tensor.transpose(pd[:], eT2048[:, sl], ident[0:8, 0:8])
            nc.vector.tensor_copy(dcolsR[:, jt * 8 : (jt + 1) * 8], pd[:])

        # Wh (node-major bf16, strided per-head layout with ones column).
        # Explicit deps keep the critical eT -> s16 -> broadcast chain ahead
        # of Wh work in the in-order Tensor/Scalar queues.
        nc.gpsimd.memset(w4[:, :, :, F], 1.0)
        for it in range(NT):
            pw = psW.tile([128, HF], f32, tag="pw")
            for kt in range(KT):
                mi = nc.tensor.matmul(
                    pw[:],
                    XTsb[:, kt * N + it * 128 : kt * N + (it + 1) * 128],
                    Wsb[:, kt * HF : (kt + 1) * HF],
                    start=(kt == 0),
                    stop=(kt == KT - 1),
                )
                if it == 0:
                    for ei in eT_mms:
                        tile.add_dep_helper(mi.ins, ei.ins, reason="eT before Wh")
            ci = nc.scalar.copy(
                w4[:, it, :, 0:F], pw[:].rearrange("p (h f) -> p h f", h=H)
            )
            if it < 4:
                for si in s16_acts:
                    tile.add_dep_helper(ci.ins, si.ins, reason="s16 before w4 copies")

    # ---------- head loop ------------------------------------------------------
    OutSB = sb.tile([128, NT * HF], f32)
    Out4 = OutSB[:].rearrange("p (it h f) -> p it h f", it=NT, h=H)
    psA = ctx.enter_context(tc.tile_pool(name="psA", bufs=2, space="PSUM"))
    pts = ctx.enter_context(tc.tile_pool(name="pts", bufs=2))
    gps = ctx.enter_context(tc.tile_pool(name="gps", bufs=2))
    eps = ctx.enter_context(tc.tile_pool(name="eps", bufs=3))
    small = ctx.enter_context(tc.tile_pool(name="small", bufs=3))

    PThs = {}

    def emit_scores_M_EXP(h, PTh, srep_idx):
        expw = 1 if h == H - 1 else 2
        for jp in range(NT // expw):
            ep = eps.tile([128, expw * N], f32, tag="ep")
            for k in range(expw):
                jt = expw * jp + k
                nc.vector._custom_dve(
                    GAT_SCORE3,
                    out=ep[:, k * N : (k + 1) * N],
                    in0=ATsb[:, jt * N : (jt + 1) * N],
                    in1=S16REP[:, srep_idx * N : (srep_idx + 1) * N],
                    s0=dcolsR[:, jt * 8 + 2 * h + 1 : jt * 8 + 2 * h + 2],
                    s1=LRELU_ALPHA,
                    imm2=-MBIG,
                )
            nc.scalar.activation(
                PTh[:, jp * expw * N : (jp + 1) * expw * N],
                ep[:],
                AF.Exp,
                scale=1.0 / SCALE,
            )

    def emit_av_norm(h, PTh):
        acc8 = psA.tile([128, NT, 128], f32, tag="acc")
        for it in range(NT):
            for jt in range(NT):
                nc.tensor.matmul(
                    acc8[:, it, 0 : F + 1],
                    PTh[:, jt * N + it * 128 : jt * N + (it + 1) * 128],
                    w4[:, jt, h, :],
                    start=(jt == 0),
                    stop=(jt == NT - 1),
                )
        ngroups = 2
        gsz = NT // ngroups
        for g in range(ngroups):
            its = slice(g * gsz, (g + 1) * gsz)
            rz = small.tile([128, gsz], f32, tag="rz")
            nc.vector.reciprocal(rz[:], acc8[:, its, F])
            u8 = small.tile([128, gsz, F], f32, tag="u8")
            nc.vector.tensor_tensor(
                u8[:], acc8[:, its, 0:F], _bcast_last(rz[:], F), op=ALU.mult
            )
            ev8 = small.tile([128, gsz, F], f32, tag="ev8")
            nc.scalar.activation(ev8[:], u8[:], AF.Exp)
            nc.vector._custom_dve(GAT_SEL2, out=Out4[:, its, h, :], in0=u8[:], in1=ev8[:])

    OutV = OutSB[:].rearrange("p (it c) -> p it c", it=NT)
    OutDV = OUTd[:].rearrange("(it p) c -> p it c", p=128)
    for h in range(H):
        PTh = pts.tile([128, NT * N], bf16, tag="pt")
        PThs[h] = PTh
        emit_scores_M_EXP(h, PTh, SREP[h])
        emit_av_norm(h, PTh)
        if h == H - 1:
            # store each it-half as soon as the last head's norm group for
            # it completes (emit_av_norm emits g0 then g1)
            nc.sync.dma_start(OutDV[:, 0 : NT // 2, :], OutV[:, 0 : NT // 2, :])
            nc.scalar.dma_start(OutDV[:, NT // 2 : NT, :], OutV[:, NT // 2 : NT, :])


# ------------------------------------------------------------------ build/run
_NC_CACHE = {}


def _build_nc():
    if "nc" in _NC_CACHE:
        return _NC_CACHE["nc"]
    nc = bacc.Bacc(
        "TRN2",
        target_bir_lowering=False,
        debug=False,
        enable_asserts=False,
        num_devices=B,
    )
    XTd = nc.dram_tensor("XT", [DIN, N], dt.float32, kind="ExternalInput").ap()
    ATd = nc.dram_tensor("A8", [N, N], dt.int8, kind="ExternalInput").ap()
    Wd = nc.dram_tensor("W", [DIN, HF], dt.float32, kind="ExternalInput").ap()
    Wad = nc.dram_tensor("Wa", [DIN, 2 * H], dt.float32, kind="ExternalInput").ap()
    ID16d = nc.dram_tensor("ID16", [16, 16], dt.float32, kind="ExternalInput").ap()
    OUTd = nc.dram_tensor("OUT", [N, HF], dt.float32, kind="ExternalOutput").ap()
    with tile.TileContext(nc) as tc:
        _gat_body(tc, XTd, ATd, Wd, Wad, ID16d, OUTd)
    nc.compile()
    _NC_CACHE["nc"] = nc
    return nc


def _host_prep(W, a_src, a_dst):
    Wh_w = np.asarray(W, np.float32).reshape(DIN, H, F)
    Wa = np.empty((DIN, 2 * H), np.float32)
    Wa[:, 0::2] = np.einsum("khf,hf->kh", Wh_w, np.asarray(a_src, np.float32))
    Wa[:, 1::2] = np.einsum("khf,hf->kh", Wh_w, np.asarray(a_dst, np.float32))
    return Wa


def _run(X, A, W, a_src, a_dst, **spmd_kwargs):
    import ml_dtypes

    bf = ml_dtypes.bfloat16
    X = np.asarray(X, np.float32)
    XT = np.ascontiguousarray(X.transpose(0, 2, 1))                  # [B, DIN, N]
    A8 = np.ascontiguousarray(
        (1 - np.asarray(A, np.int32).transpose(0, 2, 1)).astype(np.int8)
    )                                                                 # [B, N(j), N(i)]
    W = np.ascontiguousarray(np.asarray(W, np.float32))
    Wa = _host_prep(W, a_src, a_dst)
    nc = _build_nc()
    id16 = np.eye(16, dtype=np.float32)
    in_maps = [
        {"XT": XT[b], "A8": A8[b], "W": W, "Wa": Wa, "ID16": id16}
        for b in range(B)
    ]
    res = bass_utils.run_bass_kernel_spmd(
        nc, in_maps, core_ids=list(range(B)), **spmd_kwargs
    )
    out = np.stack([np.asarray(res.results[b]["OUT"]) for b in range(B)])
    return out.astype(np.float32), res


def kernel(X, A, W, a_src, a_dst):
    out, _ = _run(X, A, W, a_src, a_dst)
    return out


if __name__ == "__main__":
    rng = np.random.default_rng(0)
    out = kernel(
        X=rng.standard_normal((B, N, DIN)).astype(np.float32),
        A=rng.integers(0, 2, size=(B, N, N)).astype(np.int32),
        W=(rng.standard_normal((DIN, HF)) * 0.06).astype(np.float32),
        a_src=(rng.standard_normal((H, F)) * 0.17).astype(np.float32),
        a_dst=(rng.standard_normal((H, F)) * 0.17).astype(np.float32),
    )
    print(out.shape, out.dtype)
